# revision 22
# baseline (speedup 1.0000x reference)
"""TRN2 Bass kernel for BaseSAE forward (encode -> top-k mask -> tied decode).

Data-parallel over 8 NeuronCores: each core takes 1024 rows of x, W replicated.

Per-core phases:
  0. split x and W into fp16 (hi, lo*2048) pairs in DRAM (fp32-accurate matmul
     inputs; also enables 2-byte transpose-DMA reads).
  1. encode: z = x @ W.T (+b_enc) as 3 fp16 matmul products accumulated in
     PSUM fp32; per-row chunk-of-8 maxima M1 tracked; raw z spilled to DRAM.
  2. top-k threshold per row (exact, gather-free):
     t0 = 64th-largest chunk max (safe-shrunk), candidates = {z >= t0},
     u = -z over candidates, two max8 rounds -> 16 smallest candidates,
     cnt via ACT Sign accumulate, tau = (cnt-64)-th of them (one-hot select),
     masked z (= relu'd, exact fp32) written out + fp16 copy for decode.
  3. decode: x_hat = z_masked @ W (+b_dec) in fp16 (hi part), PSUM fp32.
"""
import numpy as np

D_IN, WIDTH, BATCH, NCORES = 2048, 16384, 8192, 8
RPC = BATCH // NCORES          # rows per core
S = 2048.0                     # lo-part scale (2^11, exact in fp16)

_CACHE = {}


def _build(has_benc, has_bdec, k, phases=(0, 1, 2, 3)):
    import concourse.bacc as bacc
    import concourse.tile as tile
    from concourse import mybir

    f32, f16 = mybir.dt.float32, mybir.dt.float16
    Alu = mybir.AluOpType
    Act = mybir.ActivationFunctionType
    KC = D_IN // 128           # 16 contraction chunks
    WB = WIDTH // 512          # 32 width blocks
    RT = RPC // 128            # 8 row tiles
    NCH = WIDTH // 8           # 2048 chunks per row
    DB = D_IN // 512           # 4 decode output blocks
    WCH = WIDTH // 128         # 128 decode contraction chunks
    rounds = (k + 7) // 8      # ladder rounds (8 for k=64)
    t0_pos = (k - 1) % 8

    nc = bacc.Bacc("TRN2", target_bir_lowering=False, debug=False,
                   num_devices=NCORES)
    in_x = nc.dram_tensor("x", [RPC, D_IN], f32, kind="ExternalInput").ap()
    in_w = nc.dram_tensor("w", [WIDTH, D_IN], f32, kind="ExternalInput").ap()
    in_be = nc.dram_tensor("be", [1, WIDTH], f32, kind="ExternalInput").ap()
    in_bd = nc.dram_tensor("bd", [1, D_IN], f32, kind="ExternalInput").ap()
    out_z = nc.dram_tensor("z_out", [RPC, WIDTH], f32, kind="ExternalOutput").ap()
    out_xh = nc.dram_tensor("xhat", [RPC, D_IN], f32, kind="ExternalOutput").ap()

    xh_d = nc.dram_tensor("xh_d", [RPC, D_IN], f16).ap()
    xl_d = nc.dram_tensor("xl_d", [RPC, D_IN], f16).ap()
    wh_d = nc.dram_tensor("wh_d", [WIDTH, D_IN], f16).ap()
    wl_d = nc.dram_tensor("wl_d", [WIDTH, D_IN], f16).ap()
    z_d = nc.dram_tensor("z_d", [RPC, WIDTH], f32).ap()
    zT_d = nc.dram_tensor("zT_d", [WIDTH, RPC], f16).ap()
    m1_d = nc.dram_tensor("m1_d", [RPC, NCH], f16).ap()

    def split_chunk(pool, nc, src_ap, dst_h, dst_l, rows=128):
        """Load fp32 chunk, emit fp16 hi + lo*S to DRAM."""
        wt = pool.tile([rows, D_IN], f32, tag="sp32")
        nc.sync.dma_start(wt[:], src_ap)
        wh = pool.tile([rows, D_IN], f16, tag="sph")
        whS = pool.tile([rows, D_IN], f16, tag="sphS")
        wl = pool.tile([rows, D_IN], f16, tag="spl")
        nc.scalar.activation(wh[:], wt[:], Act.Copy)
        nc.vector.tensor_scalar(whS[:], wh[:], S, None, Alu.mult)
        nc.vector.scalar_tensor_tensor(wl[:], wt[:], S, whS[:],
                                       Alu.mult, Alu.subtract)
        nc.sync.dma_start(dst_h, wh[:])
        nc.sync.dma_start(dst_l, wl[:])

    with tile.TileContext(nc) as tc:
        # ---------------- phase 0: split x ----------------
        if 0 in phases:
          with tc.tile_pool(name="sx", bufs=2) as sx:
            for r in range(RT):
                rs = slice(r * 128, (r + 1) * 128)
                split_chunk(sx, nc, in_x[rs, :], xh_d[rs, :], xl_d[rs, :])

        # ---------------- phase 1: encode ----------------
        if 1 in phases:
          with (
            tc.tile_pool(name="xt", bufs=1) as xt_pool,
            tc.tile_pool(name="wsp", bufs=2) as wsp,
            tc.tile_pool(name="wtp", bufs=2) as wtp,
            tc.tile_pool(name="zep", bufs=3) as zep,
            tc.tile_pool(name="eps", bufs=4, space="PSUM") as eps,
            tc.tile_pool(name="cst", bufs=1) as cst,
        ):
            if has_benc:
                ones_t = cst.tile([1, 128], f16)
                nc.vector.memset(ones_t[:], 1.0)
                be_sb = cst.tile([1, WIDTH], f32)
                nc.sync.dma_start(be_sb[:], in_be[:])

            # resident x^T fp16 tiles
            xhT, xlT = [], []
            for kk in range(KC):
                ht = xt_pool.tile([128, RPC], f16, tag=f"xhT{kk}")
                lt = xt_pool.tile([128, RPC], f16, tag=f"xlT{kk}")
                nc.sync.dma_start_transpose(ht[:], xh_d[:, kk * 128:(kk + 1) * 128])
                nc.sync.dma_start_transpose(lt[:], xl_d[:, kk * 128:(kk + 1) * 128])
                xhT.append(ht)
                xlT.append(lt)

            for wb in range(WB):
                ws = slice(wb * 512, (wb + 1) * 512)
                # split W rows for this width block
                for c in range(4):
                    rr = slice(wb * 512 + c * 128, wb * 512 + (c + 1) * 128)
                    split_chunk(wsp, nc, in_w[rr, :], wh_d[rr, :], wl_d[rr, :])
                # transposed W tiles for this block
                whT, wlT = [], []
                for kk in range(KC):
                    ks = slice(kk * 128, (kk + 1) * 128)
                    ht = wtp.tile([128, 512], f16, tag=f"whT{kk}")
                    lt = wtp.tile([128, 512], f16, tag=f"wlT{kk}")
                    nc.sync.dma_start_transpose(ht[:], wh_d[ws, ks])
                    nc.sync.dma_start_transpose(lt[:], wl_d[ws, ks])
                    whT.append(ht)
                    wlT.append(lt)

                for m in range(RT):
                    ms = slice(m * 128, (m + 1) * 128)
                    psA = eps.tile([128, 512], f32, tag="psA")
                    psB = eps.tile([128, 512], f32, tag="psB")
                    if has_benc:
                        nc.tensor.matmul(psA[:], ones_t[:], be_sb[0:1, ws],
                                         start=True, stop=False)
                    for kk in range(KC):
                        st = (kk == 0) and not has_benc
                        sp_ = kk == KC - 1
                        nc.tensor.matmul(psA[:], xhT[kk][:, ms], whT[kk][:],
                                         start=st, stop=sp_)
                        nc.tensor.matmul(psB[:], xhT[kk][:, ms], wlT[kk][:],
                                         start=(kk == 0), stop=False)
                        nc.tensor.matmul(psB[:], xlT[kk][:, ms], whT[kk][:],
                                         start=False, stop=sp_)
                    za = zep.tile([128, 512], f32, tag="za")
                    nc.scalar.activation(za[:], psA[:], Act.Copy)
                    zsb = zep.tile([128, 512], f32, tag="zsb")
                    nc.vector.scalar_tensor_tensor(zsb[:], psB[:], 1.0 / S, za[:],
                                                   Alu.mult, Alu.add)
                    m1sb = zep.tile([128, 64], f16, tag="m1sb")
                    nc.vector.tensor_reduce(
                        m1sb[:], zsb[:].rearrange("p (c e) -> p c e", e=8),
                        axis=mybir.AxisListType.X, op=Alu.max)
                    nc.sync.dma_start(z_d[ms, ws], zsb[:])
                    nc.sync.dma_start(m1_d[ms, wb * 64:(wb + 1) * 64], m1sb[:])

        # ---------------- phase 2: top-k mask ----------------
        if 2 in phases:
          with (
            tc.tile_pool(name="mk_cst", bufs=1) as mcst,
            tc.tile_pool(name="mk", bufs=2) as mk,
            tc.tile_pool(name="mks", bufs=2) as mks,
            tc.tile_pool(name="mtr", bufs=4) as mtr,
            tc.tile_pool(name="mps", bufs=4, space="PSUM") as mps,
        ):
            iotaf = mcst.tile([128, 16], f32)
            ioti = mcst.tile([128, 16], mybir.dt.int32)
            nc.gpsimd.iota(ioti[:], pattern=[[1, 16]], base=0, channel_multiplier=0)
            nc.vector.tensor_copy(iotaf[:], ioti[:])
            colid = mcst.tile([128, 128], mybir.dt.int32)
            rowid = mcst.tile([128, 128], mybir.dt.int32)
            nc.gpsimd.iota(colid[:], pattern=[[1, 128]], base=0, channel_multiplier=0)
            nc.gpsimd.iota(rowid[:], pattern=[[0, 128]], base=0, channel_multiplier=1)
            ident = mcst.tile([128, 128], f16)
            nc.vector.tensor_tensor(ident[:], colid[:], rowid[:], Alu.is_equal)

            HW_ = WIDTH // 2
            for r in range(RT):
                rs = slice(r * 128, (r + 1) * 128)
                m1 = mks.tile([128, NCH], f16, tag="m1", name=f"m1_{r}")
                nc.sync.dma_start(m1[:], m1_d[rs, :])
                r8 = mks.tile([128, 8], f16, tag="r8", name=f"r8_{r}")
                for rd in range(rounds):
                    nc.vector.max(r8[:], m1[:])
                    if rd < rounds - 1:
                        nc.vector.match_replace(m1[:], r8[:], m1[:], -1e30)
                t0p = mks.tile([128, 1], f32, tag="t0p", name=f"t0p_{r}")
                nc.vector.tensor_scalar(t0p[:], r8[:, t0_pos:t0_pos + 1],
                                        1.0 - 2.0 ** -11, None, Alu.mult)
                nt0p = mks.tile([128, 1], f32, tag="nt0p", name=f"nt0p_{r}")
                nc.vector.tensor_scalar(nt0p[:], t0p[:], -1.0, None, Alu.mult)

                zh = [None, None]
                uh = [None, None]
                cand = mks.tile([128, 32], f32, tag="cand", name=f"cand_{r}")
                cnt4 = mks.tile([128, 4], f32, tag="cnt4", name=f"cnt4_{r}")
                sgn = mks.tile([128, WIDTH // 4], f16, tag="sgn", bufs=1)
                for h in range(2):
                    hs = slice(h * HW_, (h + 1) * HW_)
                    zh[h] = mk.tile([128, HW_], f32, tag="zh", name=f"zh_{r}_{h}")
                    nc.sync.dma_start(zh[h][:], z_d[rs, hs])
                    uh[h] = mk.tile([128, HW_], f32, tag="uh", name=f"uh_{r}_{h}")
                    nc.vector.tensor_scalar(uh[h][:], zh[h][:], t0p[:], -1e30,
                                            Alu.is_lt, Alu.mult)
                    nc.vector.tensor_tensor(uh[h][:], uh[h][:], zh[h][:],
                                            Alu.subtract)
                    for q in range(2):
                        qs = slice(q * (WIDTH // 4), (q + 1) * (WIDTH // 4))
                        nc.scalar.activation(sgn[:], zh[h][:, qs], Act.Sign,
                                             bias=nt0p[:], scale=1.0,
                                             accum_out=cnt4[:, 2 * h + q:2 * h + q + 1])
                    cs = slice(h * 16, h * 16 + 8)
                    cs2 = slice(h * 16 + 8, h * 16 + 16)
                    nc.vector.max(cand[:, cs], uh[h][:])
                    nc.vector.match_replace(uh[h][:], cand[:, cs], uh[h][:], -1e30)
                    nc.vector.max(cand[:, cs2], uh[h][:])

                sreg = mks.tile([128, 1], f32, tag="sreg", name=f"sreg_{r}")
                nc.vector.tensor_reduce(sreg[:], cnt4[:],
                                        axis=mybir.AxisListType.X, op=Alu.add)
                nc.vector.tensor_scalar(sreg[:], sreg[:], float(WIDTH), 0.5,
                                        Alu.add, Alu.mult)
                nc.vector.tensor_scalar(sreg[:], sreg[:], -float(k), None,
                                        Alu.add)

                # merge halves: global top-16 of u from the 32 candidates
                u16 = mks.tile([128, 16], f32, tag="u16", name=f"u16_{r}")
                nc.vector.max(u16[:, 0:8], cand[:])
                nc.vector.match_replace(cand[:], u16[:, 0:8], cand[:], -1e30)
                nc.vector.max(u16[:, 8:16], cand[:])

                onehot = mks.tile([128, 16], f32, tag="onehot", name=f"oh_{r}")
                nc.vector.tensor_scalar(onehot[:], iotaf[:], sreg[:], None,
                                        Alu.is_equal)
                nc.vector.tensor_tensor(onehot[:], onehot[:], u16[:], Alu.mult)
                taup = mks.tile([128, 1], f32, tag="taup", name=f"taup_{r}")
                nc.vector.tensor_reduce(taup[:], onehot[:],
                                        axis=mybir.AxisListType.X, op=Alu.add)
                nc.vector.tensor_scalar(taup[:], taup[:], -1.0, 1e-30,
                                        Alu.mult, Alu.max)

                for h in range(2):
                    hs = slice(h * HW_, (h + 1) * HW_)
                    nc.vector.scalar_tensor_tensor(uh[h][:], zh[h][:], taup[:],
                                                   zh[h][:], Alu.is_ge, Alu.mult)
                    nc.sync.dma_start(out_z[rs, hs], uh[h][:])
                    zf = mks.tile([128, HW_], f16, tag="zf", name=f"zf_{r}_{h}")
                    nc.scalar.activation(zf[:], uh[h][:], Act.Copy)
                    for j in range(HW_ // 128):
                        wt = h * (HW_ // 128) + j
                        pst = mps.tile([128, 128], f16, tag="pst",
                                       name=f"pst{r}_{wt}")
                        nc.tensor.transpose(pst[:], zf[:, j * 128:(j + 1) * 128],
                                            ident[:])
                        ztr = mtr.tile([128, 128], f16, tag="ztr",
                                       name=f"ztr{r}_{wt}")
                        nc.scalar.activation(ztr[:], pst[:], Act.Copy)
                        nc.sync.dma_start(zT_d[wt * 128:(wt + 1) * 128, rs], ztr[:])

        # ---------------- phase 3: decode ----------------
        if 3 in phases:
          with (
            tc.tile_pool(name="dc_cst", bufs=1) as dcst,
            tc.tile_pool(name="dc", bufs=3) as dc,
            tc.tile_pool(name="dps", bufs=1, space="PSUM") as dps,
        ):
            if has_bdec:
                ones_d = dcst.tile([1, 128], f16)
                nc.vector.memset(ones_d[:], 1.0)
                bd_sb = dcst.tile([1, D_IN], f32)
                nc.sync.dma_start(bd_sb[:], in_bd[:])

            for db in range(DB):
                ds = slice(db * 512, (db + 1) * 512)
                pss = [dps.tile([128, 512], f32, tag=f"dp{r}", name=f"dp{db}_{r}") for r in range(RT)]
                if has_bdec:
                    for r in range(RT):
                        nc.tensor.matmul(pss[r][:], ones_d[:], bd_sb[0:1, ds],
                                         start=True, stop=False)
                for w in range(WCH):
                    wss = slice(w * 128, (w + 1) * 128)
                    zT = dc.tile([128, RPC], f16, tag="zT")
                    nc.sync.dma_start(zT[:], zT_d[wss, :])
                    wht = dc.tile([128, 512], f16, tag="wht")
                    nc.sync.dma_start(wht[:], wh_d[wss, ds])
                    for r in range(RT):
                        st = (w == 0) and not has_bdec
                        nc.tensor.matmul(pss[r][:], zT[:, r * 128:(r + 1) * 128],
                                         wht[:], start=st, stop=(w == WCH - 1))
                for r in range(RT):
                    xhsb = dc.tile([128, 512], f32, tag="xhsb")
                    nc.scalar.activation(xhsb[:], pss[r][:], Act.Copy)
                    nc.sync.dma_start(out_xh[r * 128:(r + 1) * 128, ds], xhsb[:])

    nc.compile()
    return nc


def kernel(x, W_enc, b_enc, b_dec, topk):
    from concourse.bass_utils import run_bass_kernel_spmd

    x = np.ascontiguousarray(np.asarray(x, dtype=np.float32))
    W = np.ascontiguousarray(np.asarray(W_enc, dtype=np.float32))
    be = np.asarray(b_enc, dtype=np.float32).reshape(1, -1)
    bd = np.asarray(b_dec, dtype=np.float32).reshape(1, -1)
    k = int(topk)
    key = (bool(np.any(be)), bool(np.any(bd)), k)
    if key not in _CACHE:
        _CACHE[key] = _build(*key)
    nc = _CACHE[key]

    in_maps = [
        {"x": x[i * RPC:(i + 1) * RPC], "w": W, "be": be, "bd": bd}
        for i in range(NCORES)
    ]
    res = run_bass_kernel_spmd(nc, in_maps, list(range(NCORES)))
    z = np.concatenate([res.results[i]["z_out"] for i in range(NCORES)], axis=0)
    xh = np.concatenate([res.results[i]["xhat"] for i in range(NCORES)], axis=0)
    return z, xh
